# revision 32
# baseline (speedup 1.0000x reference)
"""Trainium2 Bass kernel for nn_AttachmentPredictor.

Pipeline (per core, data-parallel over batch; 32 batches/core).

Sparsity: the reference zeroes every output where mask=0, so only
unmasked head positions (~50%) need scores at all.  The host packs each
batch's unmasked rows into a fixed-capacity slot of Q columns
(Q = max unmasked count over all batches, rounded up to 32; typically
160), giving 32*Q packed rows per core instead of 32*256.  Slot
boundaries are compile-time constants shared by all cores (SPMD), and
pad columns carry -1e9 in a log-mask so exp() kills them.

Per 512-row block:
  stage1: head projection, feature-major psum[jt] += Wh[dk,jt] @ xT[dk,:]
  tanh(Y1 + bias) -> c1, with per-(batch-slot x block) activation
  segments supplying the per-batch prep+child bias (all 32-aligned)
  stage2/3: hidden layers, feature-major, tanh -> c2, c3
  scorer: [1,512] psum rows of scores via M=1 matmuls
  epilogue: scores + logmask, exp per slot segment with accumulated
  per-slot partial sums.
Tail: combine partials, +EPS, reciprocal, scale packed exps, DMA the
packed [1, 32*Q] vector out; the host scatters to the full [B, S-2]
grid (zeros where masked).
"""

import ml_dtypes
import numpy as np

import concourse.bass as bass
import concourse.mybir as mybir
import concourse.tile as tile
from concourse import bass_utils
from concourse.bass import ts

F32 = mybir.dt.float32
F32R = mybir.dt.float32r
BF16 = mybir.dt.bfloat16
AF = mybir.ActivationFunctionType

B, S, D, P = 256, 256, 1024, 512
SH = S - 2                  # head positions per batch
NCORES = 8
BC = B // NCORES            # 32 batches per core
KD = D // 128               # 8 k-tiles over D
KP = P // 128               # 4 k-tiles over P
EPS = 1e-7
NEG = -1e9

OPTS = {
    "s1_dtype": "bf16",   # stage-1 x and Wh: "f32r" | "bf16" | "fp8dr"
    "mm_dtype": "bf16",   # stages 2/3, scorer, bias: "f32r" | "bf16"
    "xr_bufs": 4,
    "c_bufs": 10,
    "ps_bufs": 8,
}

FP8 = mybir.dt.float8e4
_DT = {"f32r": F32R, "bf16": BF16, "f32": F32, "f16": mybir.dt.float16,
       "fp8dr": FP8}
_NPDT = {"f32r": np.float32, "bf16": ml_dtypes.bfloat16, "f32": np.float32,
         "f16": np.float16, "fp8dr": ml_dtypes.float8_e4m3}
WSCALE = 64.0  # fp8 weight pre-scale: U(-.05,.05) -> +-3.2, clear of subnormals


def _slot_capacity(mask):
    """Slot width Q: max unmasked rows over all batches, rounded up to 32."""
    nb = np.asarray(mask)[:, :SH].sum(axis=1).max()
    q = max(32, int(-(-int(nb) // 32) * 32))
    return min(q, 256)


def _segments(q, nblk):
    """Per block: [(slot, lo, hi)] covering [0,512) by slot intersections."""
    segs = []
    for g in range(nblk):
        row = []
        for b in range(BC):
            lo = max(b * q, g * 512) - g * 512
            hi = min((b + 1) * q, (g + 1) * 512) - g * 512
            if lo < hi:
                row.append((b, lo, hi))
        segs.append(row)
    return segs


# ---------------------------------------------------------------------------
# walrus in this container accepts at most ONE sync wait per instruction;
# split extra waits onto preceding NoOps on the same engine.
def _split_waits(nc, maxw=1):
    ctr = 0
    for f in nc.m.functions:
        for blk in f.blocks:
            insts = blk.instructions
            newlist = []
            changed = False
            for inst in insts:
                si = inst.sync_info
                if si is not None and len(si.on_wait) > maxw:
                    waits = list(si.on_wait)
                    keep = waits[len(waits) - maxw:]
                    extra = waits[: len(waits) - maxw]
                    for j in range(0, len(extra), maxw):
                        ctr += 1
                        newlist.append(
                            mybir.InstNoOp(
                                name=f"waitsplit-{ctr}",
                                engine=inst.engine,
                                ins=[],
                                outs=[],
                                sync_info=mybir.SyncInfo(
                                    on_wait=extra[j: j + maxw], on_update=[]
                                ),
                            )
                        )
                    inst.sync_info = mybir.SyncInfo(
                        on_wait=keep, on_update=list(si.on_update)
                    )
                    changed = True
                newlist.append(inst)
            if changed:
                insts[:] = newlist


# ---------------------------------------------------------------------------
def _build(opts=None, reps=1, q=160):
    opts = dict(OPTS, **(opts or {}))
    nc = bass.Bass("TRN2", target_bir_lowering=False, debug=False)

    S1DT = _DT[opts["s1_dtype"]]
    MMDT = _DT[opts["mm_dtype"]]
    s1_fp8 = opts["s1_dtype"] == "fp8dr"
    nblk = (BC * q) // 512
    assert (BC * q) % 512 == 0
    rows = nblk * 512
    segs = _segments(q, nblk)

    # One HWDGE queue (SP), explicitly ordered by need-time: consts,
    # xr[0], wp/wc stream, xr[1], w0/w1, then the in-loop xr stream.
    s1_dma = nc.sync
    mm_dma = nc.sync
    tail_dma = nc.sync

    xT_d = nc.dram_tensor("xT", [nblk, 128, KD * 512], S1DT,
                          kind="ExternalInput").ap()
    xp_d = nc.dram_tensor("xprep", [D, BC], MMDT, kind="ExternalInput").ap()
    xc_d = nc.dram_tensor("xchild", [D, BC], MMDT, kind="ExternalInput").ap()
    if s1_fp8:
        # Wh pre-packed for DoubleRow: [dk-pair, 128, jt*256 + o*128 + m]
        wh_d = nc.dram_tensor("wh", [KD // 2, 128, 2 * P], S1DT,
                              kind="ExternalInput").ap()
    else:
        wh_d = nc.dram_tensor("wh", [D, P], S1DT, kind="ExternalInput").ap()
    wp_d = nc.dram_tensor("wp", [D, P], MMDT, kind="ExternalInput").ap()
    wc_d = nc.dram_tensor("wc", [D, P], MMDT, kind="ExternalInput").ap()
    w0_d = nc.dram_tensor("w0", [P, P], MMDT, kind="ExternalInput").ap()
    w1_d = nc.dram_tensor("w1", [P, P], MMDT, kind="ExternalInput").ap()
    sc_d = nc.dram_tensor("scT", [128, KP], MMDT, kind="ExternalInput").ap()
    lm_d = nc.dram_tensor("lmask", [1, rows], F32, kind="ExternalInput").ap()
    out_d = nc.dram_tensor("out", [BC, q], F32, kind="ExternalOutput").ap()

    with tile.TileContext(nc) as tc:
        with (
            tc.tile_pool(name="consts", bufs=1) as consts,
            tc.tile_pool(name="ssb", bufs=3) as spool,
            tc.tile_pool(name="xr", bufs=opts["xr_bufs"]) as xpool,
            tc.tile_pool(name="acts", bufs=opts["c_bufs"]) as cpool,
            tc.tile_pool(name="ps", bufs=opts["ps_bufs"], space="PSUM") as pspool,
            tc.tile_pool(name="epi", bufs=1) as epi,
        ):
            # ---- constants -------------------------------------------------
            def load_packed(dram, k, n, dt, dma, tag):
                t = consts.tile([128, k * n], dt, tag=tag)
                dma.dma_start(
                    t[:].rearrange("p (k n) -> p k n", n=n),
                    dram.rearrange("(k p) n -> p k n", p=128),
                )
                return t

            wh_t = []
            if s1_fp8:
                for dkp in range(KD // 2):
                    wt = consts.tile([128, 2 * P], S1DT, tag=f"wh{dkp}",
                                     name=f"wh_t{dkp}")
                    mm_dma.dma_start(wt[:], wh_d[dkp])
                    wh_t.append(wt)
            else:
                for dk in range(KD):
                    wt = consts.tile([128, P], S1DT, tag=f"wh{dk}",
                                     name=f"wh_t{dk}")
                    mm_dma.dma_start(wt[:], wh_d[dk * 128: (dk + 1) * 128, :])
                    wh_t.append(wt)
            xp_r = load_packed(xp_d, KD, BC, MMDT, mm_dma, "xp")
            xc_r = load_packed(xc_d, KD, BC, MMDT, mm_dma, "xc")
            sc_r = consts.tile([128, KP], MMDT, tag="sc")
            mm_dma.dma_start(sc_r[:], sc_d[:])
            lm_sb = consts.tile([1, rows], F32, tag="lm")
            mm_dma.dma_start(lm_sb[:], lm_d[:])

            exps_all = epi.tile([1, rows], F32, tag="exps")
            expsT = epi.tile([BC, q], F32, tag="expsT")
            outvT = epi.tile([BC, q], F32, tag="outvT")
            sums = epi.tile([BC, 1], F32, tag="sums")
            recips = epi.tile([BC, 1], F32, tag="recips")
            s1scale = (1.0 / WSCALE) if s1_fp8 else 1.0
            bias_fm = []

            def load_xr(g):
                xr = xpool.tile([128, KD * 512], S1DT, tag="xr")
                hw = KD * 512 // 2
                for h in range(2):
                    s1_dma.dma_start(
                        xr[:, h * hw: (h + 1) * hw],
                        xT_d[g, :, h * hw: (h + 1) * hw],
                    )
                return xr

            def s1_mms(g, xr):
                pss1s = []
                for jt in range(KP):
                    pss1 = pspool.tile([128, 512], F32, tag="ps",
                                       name=f"ps1_{g}_{jt}")
                    if s1_fp8:
                        for dkp in range(KD // 2):
                            nc.tensor.matmul(
                                pss1[:],
                                wh_t[dkp][:, jt * 256: (jt + 1) * 256]
                                .rearrange("p (o m) -> p o m", o=2),
                                xr[:, dkp * 1024: (dkp + 1) * 1024]
                                .rearrange("p (o n) -> p o n", o=2),
                                start=(dkp == 0),
                                stop=(dkp == KD // 2 - 1),
                                perf_mode=mybir.MatmulPerfMode.DoubleRow,
                            )
                    else:
                        for dk in range(KD):
                            nc.tensor.matmul(
                                pss1[:],
                                wh_t[dk][:, jt * 128: (jt + 1) * 128],
                                xr[:, dk * 512: (dk + 1) * 512],
                                start=(dk == 0),
                                stop=(dk == KD - 1),
                            )
                    pss1s.append(pss1)
                return pss1s

            def block_rest(g, pss1s):
                # per-batch bias added in-place on the (mostly idle) DVE
                # (gpsimd cannot access PSUM), then ONE full-tile tanh on
                # ACT: the ACT instruction has a ~293ns fixed cost, so
                # fewer, larger activations win.
                c_in = [None] * KP
                for jt in range(KP):
                    pss1 = pss1s[jt]
                    for (b, lo, hi) in segs[g]:
                        nc.vector.tensor_scalar_add(
                            pss1[:, lo:hi], pss1[:, lo:hi],
                            bias_fm[jt][:, b: b + 1],
                        )
                    ct = cpool.tile([128, 512], MMDT, tag="c1")
                    nc.scalar.activation(ct[:], pss1[:], AF.Tanh,
                                         scale=s1scale)
                    c_in[jt] = ct

                for stage, w_r in ((2, w0_r), (3, w1_r)):
                    c_out = [None] * KP
                    for qt in range(KP):
                        pss2 = pspool.tile([128, 512], F32, tag="ps",
                                           name=f"ps{stage}_{g}_{qt}")
                        for jk in range(KP):
                            nc.tensor.matmul(
                                pss2[:],
                                w_r[:, jk * P + qt * 128:
                                    jk * P + (qt + 1) * 128],
                                c_in[jk][:],
                                start=(jk == 0),
                                stop=(jk == KP - 1),
                            )
                        ct = cpool.tile([128, 512], MMDT, tag=f"c{stage}")
                        nc.scalar.activation(ct[:], pss2[:], AF.Tanh)
                        c_out[qt] = ct
                    c_in = c_out

                # scorer -> [1, 512] scores; pad log-mask added in-place on
                # DVE (PE is the bottleneck, DVE is nearly idle).
                pss = pspool.tile([1, 512], F32, tag="ps", name=f"pssc_{g}")
                for qk in range(KP):
                    nc.tensor.matmul(
                        pss[:],
                        sc_r[:, qk: qk + 1],
                        c_in[qk][:],
                        start=(qk == 0),
                        stop=(qk == KP - 1),
                    )
                nc.vector.tensor_add(
                    pss[0:1, :], pss[0:1, :],
                    lm_sb[0:1, g * 512: (g + 1) * 512],
                )
                nc.scalar.activation(
                    exps_all[0:1, g * 512: (g + 1) * 512],
                    pss[0:1, :],
                    AF.Exp,
                )

            # ---- prologue, ordered by need-time ---------------------------
            # xr[0] + block-0 stage-1 MMs go FIRST (PE is FIFO: anything
            # emitted earlier that waits on a late DMA head-of-line blocks
            # the whole engine).  Then the bias-prep MMs (wp/wc stream),
            # then xr[1], then w0/w1.
            xr0 = load_xr(0)
            pre_pss1 = s1_mms(0, xr0)

            psbs = [pspool.tile([128, BC], F32, tag="ps", name=f"psb_{jt}")
                    for jt in range(KP)]
            for i, (xs, w_d) in enumerate(((xp_r, wp_d), (xc_r, wc_d))):
                for dk in range(KD):
                    wst = spool.tile([128, P], MMDT, tag="wst")
                    mm_dma.dma_start(wst[:], w_d[dk * 128: (dk + 1) * 128, :])
                    for jt in range(KP):
                        nc.tensor.matmul(
                            psbs[jt][:],
                            wst[:, jt * 128: (jt + 1) * 128],
                            xs[:, dk * BC: (dk + 1) * BC],
                            start=(i == 0 and dk == 0),
                            stop=(i == 1 and dk == KD - 1),
                        )
            for jt in range(KP):
                bf = consts.tile([128, BC], F32, tag=f"bias{jt}")
                if s1_fp8:
                    # bias is now added pre-scale: tanh(scale*(psum + b*WS))
                    nc.vector.tensor_scalar_mul(bf[:], psbs[jt][:], WSCALE)
                else:
                    nc.vector.tensor_copy(bf[:], psbs[jt][:])
                bias_fm.append(bf)

            xr1 = load_xr(1) if nblk > 1 else None
            w0_r = load_packed(w0_d, KP, P, MMDT, mm_dma, "w0")
            w1_r = load_packed(w1_d, KP, P, MMDT, mm_dma, "w1")

            # ---- main loop -------------------------------------------------
            for _rep in range(reps):
                for g in range(nblk):
                    if _rep == 0 and g == 0:
                        pss1s = pre_pss1
                    else:
                        if _rep == 0 and g == 1:
                            xr = xr1
                        else:
                            xr = load_xr(g)
                        pss1s = s1_mms(g, xr)
                    block_rest(g, pss1s)

                # ---- tail: [1, 32*q] -> [32, q], rowsum, scale, DMA -------
                tail_dma.dma_start(
                    expsT[:],
                    exps_all[:].rearrange("p (b c) -> p b c", c=q),
                )
                nc.vector.tensor_reduce(sums[:], expsT[:],
                                        mybir.AxisListType.X,
                                        mybir.AluOpType.add)
                nc.vector.tensor_scalar_add(sums[:], sums[:], EPS)
                nc.vector.reciprocal(recips[:], sums[:])
                nc.vector.tensor_scalar_mul(outvT[:], expsT[:],
                                            recips[:, 0:1])
                tail_dma.dma_start(out_d[:], outvT[:])

    _split_waits(nc)
    return nc


# ---------------------------------------------------------------------------
class _Plan:
    def __init__(self, q, idx_lists):
        self.q = q
        self.nblk = (BC * q) // 512
        self.idx_lists = idx_lists  # [B] arrays of unmasked positions


def _host_prep(x, proj_head, proj_prep, proj_child, hidden_layers, scorer,
               mask, opts=None):
    opts = dict(OPTS, **(opts or {}))
    s1_np = _NPDT[opts["s1_dtype"]]
    mm_np = _NPDT[opts["mm_dtype"]]
    x = np.asarray(x, np.float32)
    mask = np.asarray(mask)
    q = _slot_capacity(mask)
    nblk = (BC * q) // 512
    rows = nblk * 512
    idx_lists = [np.nonzero(mask[b, :SH])[0] for b in range(B)]

    s1_fp8 = opts["s1_dtype"] == "fp8dr"
    whf = np.asarray(proj_head, np.float32)
    if s1_fp8:
        wh = np.ascontiguousarray(
            np.clip(whf * WSCALE, -240, 240)
            .reshape(KD // 2, 2, 128, KP, 128).transpose(0, 2, 3, 1, 4)
            .reshape(KD // 2, 128, 2 * P).astype(s1_np)
        )
    else:
        wh = np.ascontiguousarray(whf.astype(s1_np))
    wp = np.ascontiguousarray(np.asarray(proj_prep, mm_np))
    wc = np.ascontiguousarray(np.asarray(proj_child, mm_np))
    hl = np.asarray(hidden_layers, np.float32)
    w0 = np.ascontiguousarray(hl[0].astype(mm_np))
    w1 = np.ascontiguousarray(hl[1].astype(mm_np))
    scT = np.ascontiguousarray(
        np.asarray(scorer, np.float32).reshape(KP, 128).T.astype(mm_np)
    )  # [128, 4]

    in_maps = []
    for c in range(NCORES):
        xb = x[c * BC: (c + 1) * BC]                        # [32, 256, 1024]
        packed = np.zeros((rows, D), np.float32)
        lm = np.full((1, rows), NEG, np.float32)
        for b in range(BC):
            idx = idx_lists[c * BC + b]
            n = len(idx)
            packed[b * q: b * q + n] = xb[b, idx, :]
            lm[0, b * q: b * q + n] = 0.0
        if s1_fp8:
            xTc = np.ascontiguousarray(
                packed.reshape(nblk, 512, KD // 2, 2, 128)
                .transpose(0, 4, 2, 3, 1).astype(s1_np)
            ).reshape(nblk, 128, KD * 512)
        else:
            xTc = np.ascontiguousarray(
                packed.reshape(nblk, 512, KD, 128).transpose(0, 3, 2, 1)
                .astype(s1_np)
            ).reshape(nblk, 128, KD * 512)
        xpc = np.ascontiguousarray(xb[:, S - 2, :].T.astype(mm_np))  # [1024,32]
        xcc = np.ascontiguousarray(xb[:, S - 1, :].T.astype(mm_np))  # [1024,32]
        in_maps.append(
            {
                "xT": xTc, "xprep": xpc, "xchild": xcc,
                "wh": wh, "wp": wp, "wc": wc, "w0": w0, "w1": w1,
                "scT": scT, "lmask": lm,
            }
        )
    return in_maps, _Plan(q, idx_lists)


_NC_CACHE = {}


def _get_nc(q, opts=None, reps=1):
    key = (q, reps, tuple(sorted((opts or {}).items())))
    if key not in _NC_CACHE:
        _NC_CACHE[key] = _build(opts=opts, reps=reps, q=q)
    return _NC_CACHE[key]


def kernel(x, proj_head, proj_prep, proj_child, hidden_layers, scorer, mask,
           opts=None):
    in_maps, plan = _host_prep(
        x, proj_head, proj_prep, proj_child, hidden_layers, scorer, mask,
        opts=opts,
    )
    nc = _get_nc(plan.q, opts=opts)
    res = bass_utils.run_bass_kernel_spmd(
        nc, in_maps, core_ids=list(range(NCORES))
    )
    out = np.zeros((B, SH), np.float32)
    for c in range(NCORES):
        vals = res.results[c]["out"]            # [BC, q]
        for b in range(BC):
            idx = plan.idx_lists[c * BC + b]
            out[c * BC + b, idx] = vals[b, : len(idx)]
    return out


if __name__ == "__main__":
    rng = np.random.default_rng(0)
    x = rng.standard_normal((B, S, D)).astype(np.float32)
    u = lambda shp: rng.uniform(-0.05, 0.05, shp).astype(np.float32)
    inputs = dict(
        x=x, proj_head=u((D, P)), proj_prep=u((D, P)), proj_child=u((D, P)),
        hidden_layers=u((2, P, P)), scorer=u((P,)),
        mask=rng.integers(0, 2, (B, S)).astype(bool),
    )
    out = kernel(**inputs)
    print("kernel out", out.shape, out.dtype, out[:2, :4])


# revision 46
# speedup vs baseline: 1.0077x; 1.0077x over previous
"""Trainium2 Bass kernel for nn_AttachmentPredictor.

Pipeline (per core, data-parallel over batch; 32 batches/core).

Sparsity: the reference zeroes every output where mask=0, so only
unmasked head positions (~50%) need scores at all.  The host packs each
batch's unmasked rows into a fixed-capacity slot of Q columns
(Q = max unmasked count over all batches, rounded up to 32; typically
160), giving 32*Q packed rows per core instead of 32*256.  Slot
boundaries are compile-time constants shared by all cores (SPMD), and
pad columns carry -1e9 in a log-mask so exp() kills them.

Per 512-row block:
  stage1: head projection, feature-major psum[jt] += Wh[dk,jt] @ xT[dk,:]
  tanh(Y1 + bias) -> c1, with per-(batch-slot x block) activation
  segments supplying the per-batch prep+child bias (all 32-aligned)
  stage2/3: hidden layers, feature-major, tanh -> c2, c3
  scorer: [1,512] psum rows of scores via M=1 matmuls
  epilogue: scores + logmask, exp per slot segment with accumulated
  per-slot partial sums.
Tail: combine partials, +EPS, reciprocal, scale packed exps, DMA the
packed [1, 32*Q] vector out; the host scatters to the full [B, S-2]
grid (zeros where masked).
"""

import ml_dtypes
import numpy as np

import concourse.bass as bass
import concourse.mybir as mybir
import concourse.tile as tile
from concourse import bass_utils
from concourse.bass import ts

F32 = mybir.dt.float32
F32R = mybir.dt.float32r
BF16 = mybir.dt.bfloat16
AF = mybir.ActivationFunctionType

B, S, D, P = 256, 256, 1024, 512
SH = S - 2                  # head positions per batch
NCORES = 8
BC = B // NCORES            # 32 batches per core
KD = D // 128               # 8 k-tiles over D
KP = P // 128               # 4 k-tiles over P
EPS = 1e-7
NEG = -1e9

OPTS = {
    "s1_dtype": "bf16",   # stage-1 x and Wh: "f32r" | "bf16" | "fp8dr"
    "mm_dtype": "bf16",   # stages 2/3, scorer, bias: "f32r" | "bf16"
    "xr_bufs": 4,
    "c_bufs": 10,
    "ps_bufs": 8,
}

FP8 = mybir.dt.float8e4
_DT = {"f32r": F32R, "bf16": BF16, "f32": F32, "f16": mybir.dt.float16,
       "fp8dr": FP8}
_NPDT = {"f32r": np.float32, "bf16": ml_dtypes.bfloat16, "f32": np.float32,
         "f16": np.float16, "fp8dr": ml_dtypes.float8_e4m3}
WSCALE = 64.0  # fp8 weight pre-scale: U(-.05,.05) -> +-3.2, clear of subnormals


def _slot_plan(mask):
    """Per-rank slot sizes: each core's batches are sorted by unmasked
    count (descending) and slot k is sized to the max k-th-largest count
    over all cores (rounded up to 8).  Compile-time-uniform across cores,
    ~15% fewer rows than a single global max slot width."""
    n = np.asarray(mask)[:, :SH].sum(axis=1).reshape(NCORES, BC)
    srt = -np.sort(-n, axis=1)                  # [cores, BC] desc
    s = np.maximum(srt.max(axis=0), 1)
    s = ((s + 7) // 8 * 8).astype(int)
    assert s.max() <= 512
    return tuple(int(v) for v in s)


def _bounds(slots):
    b = [0]
    for s in slots:
        b.append(b[-1] + s)
    return b


def _segments(slots, nblk):
    """Per block: [(slot, lo, hi)] intersections of slot ranges with
    [512g, 512(g+1)), block-local coords."""
    bnd = _bounds(slots)
    segs = []
    for g in range(nblk):
        row = []
        for b in range(BC):
            lo = max(bnd[b], g * 512) - g * 512
            hi = min(bnd[b + 1], (g + 1) * 512) - g * 512
            if lo < hi:
                row.append((b, lo, hi))
        segs.append(row)
    return segs


# ---------------------------------------------------------------------------
# walrus in this container accepts at most ONE sync wait per instruction;
# split extra waits onto preceding NoOps on the same engine.
def _split_waits(nc, maxw=1):
    ctr = 0
    for f in nc.m.functions:
        for blk in f.blocks:
            insts = blk.instructions
            newlist = []
            changed = False
            for inst in insts:
                si = inst.sync_info
                if si is not None and len(si.on_wait) > maxw:
                    waits = list(si.on_wait)
                    keep = waits[len(waits) - maxw:]
                    extra = waits[: len(waits) - maxw]
                    for j in range(0, len(extra), maxw):
                        ctr += 1
                        newlist.append(
                            mybir.InstNoOp(
                                name=f"waitsplit-{ctr}",
                                engine=inst.engine,
                                ins=[],
                                outs=[],
                                sync_info=mybir.SyncInfo(
                                    on_wait=extra[j: j + maxw], on_update=[]
                                ),
                            )
                        )
                    inst.sync_info = mybir.SyncInfo(
                        on_wait=keep, on_update=list(si.on_update)
                    )
                    changed = True
                newlist.append(inst)
            if changed:
                insts[:] = newlist


# ---------------------------------------------------------------------------
def _build(opts=None, reps=1, slots=None):
    opts = dict(OPTS, **(opts or {}))
    nc = bass.Bass("TRN2", target_bir_lowering=False, debug=False)

    S1DT = _DT[opts["s1_dtype"]]
    MMDT = _DT[opts["mm_dtype"]]
    s1_fp8 = opts["s1_dtype"] == "fp8dr"
    slots = tuple(slots) if slots else (160,) * BC
    nblk = -(-sum(slots) // 512)
    rows = nblk * 512
    segs = _segments(slots, nblk)

    # One HWDGE queue (SP), explicitly ordered by need-time: consts,
    # xr[0], wp/wc stream, xr[1], w0/w1, then the in-loop xr stream.
    s1_dma = nc.sync
    mm_dma = nc.sync
    tail_dma = nc.sync

    xT_d = nc.dram_tensor("xT", [nblk, 128, KD * 512], S1DT,
                          kind="ExternalInput").ap()
    xp_d = nc.dram_tensor("xprep", [D, BC], MMDT, kind="ExternalInput").ap()
    xc_d = nc.dram_tensor("xchild", [D, BC], MMDT, kind="ExternalInput").ap()
    if s1_fp8:
        # Wh pre-packed for DoubleRow: [dk-pair, 128, jt*256 + o*128 + m]
        wh_d = nc.dram_tensor("wh", [KD // 2, 128, 2 * P], S1DT,
                              kind="ExternalInput").ap()
    else:
        wh_d = nc.dram_tensor("wh", [D, P], S1DT, kind="ExternalInput").ap()
    wp_d = nc.dram_tensor("wp", [D, P], MMDT, kind="ExternalInput").ap()
    wc_d = nc.dram_tensor("wc", [D, P], MMDT, kind="ExternalInput").ap()
    w0_d = nc.dram_tensor("w0", [P, P], MMDT, kind="ExternalInput").ap()
    w1_d = nc.dram_tensor("w1", [P, P], MMDT, kind="ExternalInput").ap()
    sc_d = nc.dram_tensor("scT", [128, KP], MMDT, kind="ExternalInput").ap()
    lm_d = nc.dram_tensor("lmask", [1, rows], F32, kind="ExternalInput").ap()
    out_d = nc.dram_tensor("out", [1, rows], F32, kind="ExternalOutput").ap()

    with tile.TileContext(nc) as tc:
        with (
            tc.tile_pool(name="consts", bufs=1) as consts,
            tc.tile_pool(name="ssb", bufs=3) as spool,
            tc.tile_pool(name="xr", bufs=opts["xr_bufs"]) as xpool,
            tc.tile_pool(name="acts", bufs=opts["c_bufs"]) as cpool,
            tc.tile_pool(name="ps", bufs=opts["ps_bufs"], space="PSUM") as pspool,
            tc.tile_pool(name="epi", bufs=1) as epi,
        ):
            # ---- constants -------------------------------------------------
            def load_packed(dram, k, n, dt, dma, tag):
                t = consts.tile([128, k * n], dt, tag=tag)
                dma.dma_start(
                    t[:].rearrange("p (k n) -> p k n", n=n),
                    dram.rearrange("(k p) n -> p k n", p=128),
                )
                return t

            wh_t = []
            if s1_fp8:
                for dkp in range(KD // 2):
                    wt = consts.tile([128, 2 * P], S1DT, tag=f"wh{dkp}",
                                     name=f"wh_t{dkp}")
                    mm_dma.dma_start(wt[:], wh_d[dkp])
                    wh_t.append(wt)
            else:
                for dk in range(KD):
                    wt = consts.tile([128, P], S1DT, tag=f"wh{dk}",
                                     name=f"wh_t{dk}")
                    mm_dma.dma_start(wt[:], wh_d[dk * 128: (dk + 1) * 128, :])
                    wh_t.append(wt)
            xp_r = load_packed(xp_d, KD, BC, MMDT, mm_dma, "xp")
            xc_r = load_packed(xc_d, KD, BC, MMDT, mm_dma, "xc")
            sc_r = consts.tile([128, KP], MMDT, tag="sc")
            mm_dma.dma_start(sc_r[:], sc_d[:])
            lm_sb = consts.tile([1, rows], F32, tag="lm")
            mm_dma.dma_start(lm_sb[:], lm_d[:])

            exps_all = epi.tile([1, rows], F32, tag="exps")
            outv = epi.tile([1, rows], F32, tag="outv")
            parts = epi.tile([1, 2 * BC], F32, tag="parts")
            sums = epi.tile([1, BC], F32, tag="sums")
            recips = epi.tile([1, BC], F32, tag="recips")
            s1scale = (1.0 / WSCALE) if s1_fp8 else 1.0
            bias_fm = []
            occ = {}

            def load_xr(g):
                xr = xpool.tile([128, KD * 512], S1DT, tag="xr")
                hw = KD * 512 // 2
                for h in range(2):
                    s1_dma.dma_start(
                        xr[:, h * hw: (h + 1) * hw],
                        xT_d[g, :, h * hw: (h + 1) * hw],
                    )
                return xr

            def s1_mms(g, xr):
                pss1s = []
                for jt in range(KP):
                    pss1 = pspool.tile([128, 512], F32, tag="ps",
                                       name=f"ps1_{g}_{jt}")
                    if s1_fp8:
                        for dkp in range(KD // 2):
                            nc.tensor.matmul(
                                pss1[:],
                                wh_t[dkp][:, jt * 256: (jt + 1) * 256]
                                .rearrange("p (o m) -> p o m", o=2),
                                xr[:, dkp * 1024: (dkp + 1) * 1024]
                                .rearrange("p (o n) -> p o n", o=2),
                                start=(dkp == 0),
                                stop=(dkp == KD // 2 - 1),
                                perf_mode=mybir.MatmulPerfMode.DoubleRow,
                            )
                    else:
                        for dk in range(KD):
                            nc.tensor.matmul(
                                pss1[:],
                                wh_t[dk][:, jt * 128: (jt + 1) * 128],
                                xr[:, dk * 512: (dk + 1) * 512],
                                start=(dk == 0),
                                stop=(dk == KD - 1),
                            )
                    pss1s.append(pss1)
                return pss1s

            def block_rest(g, pss1s):
                # per-batch bias added in-place on the (mostly idle) DVE
                # (gpsimd cannot access PSUM), then ONE full-tile tanh on
                # ACT: the ACT instruction has a ~293ns fixed cost, so
                # fewer, larger activations win.
                c_in = [None] * KP
                for jt in range(KP):
                    pss1 = pss1s[jt]
                    for (b, lo, hi) in segs[g]:
                        nc.vector.tensor_scalar_add(
                            pss1[:, lo:hi], pss1[:, lo:hi],
                            bias_fm[jt][:, b: b + 1],
                        )
                    ct = cpool.tile([128, 512], MMDT, tag="c1")
                    nc.scalar.activation(ct[:], pss1[:], AF.Tanh,
                                         scale=s1scale)
                    c_in[jt] = ct

                for stage, w_r in ((2, w0_r), (3, w1_r)):
                    c_out = [None] * KP
                    for qt in range(KP):
                        pss2 = pspool.tile([128, 512], F32, tag="ps",
                                           name=f"ps{stage}_{g}_{qt}")
                        for jk in range(KP):
                            nc.tensor.matmul(
                                pss2[:],
                                w_r[:, jk * P + qt * 128:
                                    jk * P + (qt + 1) * 128],
                                c_in[jk][:],
                                start=(jk == 0),
                                stop=(jk == KP - 1),
                            )
                        ct = cpool.tile([128, 512], MMDT, tag=f"c{stage}")
                        nc.scalar.activation(ct[:], pss2[:], AF.Tanh)
                        c_out[qt] = ct
                    c_in = c_out

                # scorer -> [1, 512] scores; pad log-mask added in-place on
                # DVE (PE is the bottleneck, DVE is nearly idle).
                pss = pspool.tile([1, 512], F32, tag="ps", name=f"pssc_{g}")
                for qk in range(KP):
                    nc.tensor.matmul(
                        pss[:],
                        sc_r[:, qk: qk + 1],
                        c_in[qk][:],
                        start=(qk == 0),
                        stop=(qk == KP - 1),
                    )
                nc.vector.tensor_add(
                    pss[0:1, :], pss[0:1, :],
                    lm_sb[0:1, g * 512: (g + 1) * 512],
                )
                nc.scalar.activation(
                    exps_all[0:1, g * 512: (g + 1) * 512],
                    pss[0:1, :],
                    AF.Exp,
                )
                # per-slot partial sums on DVE (pads contribute exp(-1e9)=0)
                for (b, lo, hi) in segs[g]:
                    k = occ.get(b, 0)
                    occ[b] = k + 1
                    assert k < 2, "slot spans >2 blocks"
                    nc.vector.tensor_reduce(
                        parts[0:1, k * BC + b: k * BC + b + 1],
                        exps_all[0:1, g * 512 + lo: g * 512 + hi],
                        mybir.AxisListType.X,
                        mybir.AluOpType.add,
                    )

            # ---- prologue, ordered by need-time ---------------------------
            # xr[0] + block-0 stage-1 MMs go FIRST (PE is FIFO: anything
            # emitted earlier that waits on a late DMA head-of-line blocks
            # the whole engine).  Then the bias-prep MMs (wp/wc stream),
            # then xr[1], then w0/w1.
            xr0 = load_xr(0)
            pre_pss1 = s1_mms(0, xr0)

            psbs = [pspool.tile([128, BC], F32, tag="ps", name=f"psb_{jt}")
                    for jt in range(KP)]
            for i, (xs, w_d) in enumerate(((xp_r, wp_d), (xc_r, wc_d))):
                for dk in range(KD):
                    wst = spool.tile([128, P], MMDT, tag="wst")
                    mm_dma.dma_start(wst[:], w_d[dk * 128: (dk + 1) * 128, :])
                    for jt in range(KP):
                        nc.tensor.matmul(
                            psbs[jt][:],
                            wst[:, jt * 128: (jt + 1) * 128],
                            xs[:, dk * BC: (dk + 1) * BC],
                            start=(i == 0 and dk == 0),
                            stop=(i == 1 and dk == KD - 1),
                        )
            for jt in range(KP):
                bf = consts.tile([128, BC], F32, tag=f"bias{jt}")
                if s1_fp8:
                    # bias is now added pre-scale: tanh(scale*(psum + b*WS))
                    nc.vector.tensor_scalar_mul(bf[:], psbs[jt][:], WSCALE)
                else:
                    nc.vector.tensor_copy(bf[:], psbs[jt][:])
                bias_fm.append(bf)

            xr1 = load_xr(1) if nblk > 1 else None
            w0_r = load_packed(w0_d, KP, P, MMDT, mm_dma, "w0")
            w1_r = load_packed(w1_d, KP, P, MMDT, mm_dma, "w1")

            # ---- main loop -------------------------------------------------
            for _rep in range(reps):
                occ.clear()
                nc.gpsimd.memset(parts[:], 0.0)
                for g in range(nblk):
                    if _rep == 0 and g == 0:
                        pss1s = pre_pss1
                    else:
                        if _rep == 0 and g == 1:
                            xr = xr1
                        else:
                            xr = load_xr(g)
                        pss1s = s1_mms(g, xr)
                    block_rest(g, pss1s)

                # ---- tail: combine partials, normalize, scale, DMA --------
                nc.vector.tensor_add(sums[:], parts[0:1, 0:BC],
                                     parts[0:1, BC: 2 * BC])
                nc.vector.tensor_scalar_add(sums[:], sums[:], EPS)
                nc.vector.reciprocal(recips[:], sums[:])
                # scalings on the idle Pool engine (all-SBUF operands) so
                # the DVE stays free for the next rep's bias adds
                for g in range(nblk):
                    for (b, lo, hi) in segs[g]:
                        nc.gpsimd.tensor_scalar_mul(
                            outv[0:1, g * 512 + lo: g * 512 + hi],
                            exps_all[0:1, g * 512 + lo: g * 512 + hi],
                            recips[0:1, b: b + 1],
                        )
                tail_dma.dma_start(out_d[:], outv[:])

    _split_waits(nc)
    return nc


# ---------------------------------------------------------------------------
class _Plan:
    def __init__(self, slots, idx_lists, orders):
        self.slots = slots
        self.bounds = _bounds(slots)
        self.nblk = -(-sum(slots) // 512)
        self.idx_lists = idx_lists  # [B] arrays of unmasked positions
        self.orders = orders        # [cores][BC] slot -> local batch index


def _host_prep(x, proj_head, proj_prep, proj_child, hidden_layers, scorer,
               mask, opts=None):
    opts = dict(OPTS, **(opts or {}))
    s1_np = _NPDT[opts["s1_dtype"]]
    mm_np = _NPDT[opts["mm_dtype"]]
    x = np.asarray(x, np.float32)
    mask = np.asarray(mask)
    idx_lists = [np.nonzero(mask[b, :SH])[0] for b in range(B)]
    nb = np.array([len(i) for i in idx_lists]).reshape(NCORES, BC)
    orders = [list(np.argsort(-nb[c], kind="stable")) for c in range(NCORES)]
    slots = _slot_plan(mask)
    bounds = _bounds(slots)
    nblk = -(-sum(slots) // 512)
    rows = nblk * 512

    s1_fp8 = opts["s1_dtype"] == "fp8dr"
    whf = np.asarray(proj_head, np.float32)
    if s1_fp8:
        wh = np.ascontiguousarray(
            np.clip(whf * WSCALE, -240, 240)
            .reshape(KD // 2, 2, 128, KP, 128).transpose(0, 2, 3, 1, 4)
            .reshape(KD // 2, 128, 2 * P).astype(s1_np)
        )
    else:
        wh = np.ascontiguousarray(whf.astype(s1_np))
    wp = np.ascontiguousarray(np.asarray(proj_prep, mm_np))
    wc = np.ascontiguousarray(np.asarray(proj_child, mm_np))
    hl = np.asarray(hidden_layers, np.float32)
    w0 = np.ascontiguousarray(hl[0].astype(mm_np))
    w1 = np.ascontiguousarray(hl[1].astype(mm_np))
    scT = np.ascontiguousarray(
        np.asarray(scorer, np.float32).reshape(KP, 128).T.astype(mm_np)
    )  # [128, 4]

    in_maps = []
    for c in range(NCORES):
        xb = x[c * BC: (c + 1) * BC]                        # [32, 256, 1024]
        packed = np.zeros((rows, D), np.float32)
        lm = np.full((1, rows), NEG, np.float32)
        for k in range(BC):                # slot k holds batch orders[c][k]
            b = orders[c][k]
            idx = idx_lists[c * BC + b]
            n = len(idx)
            assert n <= slots[k]
            packed[bounds[k]: bounds[k] + n] = xb[b, idx, :]
            lm[0, bounds[k]: bounds[k] + n] = 0.0
        if s1_fp8:
            xTc = np.ascontiguousarray(
                packed.reshape(nblk, 512, KD // 2, 2, 128)
                .transpose(0, 4, 2, 3, 1).astype(s1_np)
            ).reshape(nblk, 128, KD * 512)
        else:
            xTc = np.ascontiguousarray(
                packed.reshape(nblk, 512, KD, 128).transpose(0, 3, 2, 1)
                .astype(s1_np)
            ).reshape(nblk, 128, KD * 512)
        # prep/child encodings in SLOT order so bias_fm columns line up
        xpc = np.ascontiguousarray(
            xb[orders[c], S - 2, :].T.astype(mm_np))             # [1024, 32]
        xcc = np.ascontiguousarray(
            xb[orders[c], S - 1, :].T.astype(mm_np))             # [1024, 32]
        in_maps.append(
            {
                "xT": xTc, "xprep": xpc, "xchild": xcc,
                "wh": wh, "wp": wp, "wc": wc, "w0": w0, "w1": w1,
                "scT": scT, "lmask": lm,
            }
        )
    return in_maps, _Plan(slots, idx_lists, orders)


_NC_CACHE = {}


def _get_nc(slots, opts=None, reps=1):
    key = (tuple(slots), reps, tuple(sorted((opts or {}).items())))
    if key not in _NC_CACHE:
        _NC_CACHE[key] = _build(opts=opts, reps=reps, slots=slots)
    return _NC_CACHE[key]


def kernel(x, proj_head, proj_prep, proj_child, hidden_layers, scorer, mask,
           opts=None):
    in_maps, plan = _host_prep(
        x, proj_head, proj_prep, proj_child, hidden_layers, scorer, mask,
        opts=opts,
    )
    nc = _get_nc(plan.slots, opts=opts)
    res = bass_utils.run_bass_kernel_spmd(
        nc, in_maps, core_ids=list(range(NCORES))
    )
    out = np.zeros((B, SH), np.float32)
    for c in range(NCORES):
        vals = res.results[c]["out"][0]         # [1, rows] packed
        for k in range(BC):
            b = plan.orders[c][k]
            idx = plan.idx_lists[c * BC + b]
            out[c * BC + b, idx] = vals[plan.bounds[k]:
                                        plan.bounds[k] + len(idx)]
    return out


if __name__ == "__main__":
    rng = np.random.default_rng(0)
    x = rng.standard_normal((B, S, D)).astype(np.float32)
    u = lambda shp: rng.uniform(-0.05, 0.05, shp).astype(np.float32)
    inputs = dict(
        x=x, proj_head=u((D, P)), proj_prep=u((D, P)), proj_child=u((D, P)),
        hidden_layers=u((2, P, P)), scorer=u((P,)),
        mask=rng.integers(0, 2, (B, S)).astype(bool),
    )
    out = kernel(**inputs)
    print("kernel out", out.shape, out.dtype, out[:2, :4])


# revision 47
# speedup vs baseline: 1.1447x; 1.1359x over previous
"""Trainium2 Bass kernel for nn_AttachmentPredictor.

Pipeline (per core, data-parallel over batch; 32 batches/core).

Sparsity: the reference zeroes every output where mask=0, so only
unmasked head positions (~50%) need scores at all.  The host sorts each
core's batches by unmasked count (descending) and packs batch rank k
into a slot sized to the max k-th-largest count over all cores (rounded
to 8) — compile-time-uniform across cores (SPMD) yet ~18% tighter than
one global slot width (typ. ~4224 rows/core = 9 blocks vs 8192 dense).
Pad columns carry -1e9 in a log-mask so exp() kills them.  The NEFF is
(re)compiled per slot plan and cached; any mask distribution works.

Per 512-row block:
  stage1: head projection, feature-major psum[jt] += Wh[dk,jt] @ xT[dk,:]
  tanh(Y1 + bias) -> c1, with per-(batch-slot x block) activation
  segments supplying the per-batch prep+child bias (all 32-aligned)
  stage2/3: hidden layers, feature-major, tanh -> c2, c3
  scorer: [1,512] psum rows of scores via M=1 matmuls
  epilogue: scores + logmask, exp per slot segment with accumulated
  per-slot partial sums.
Tail: combine partials, +EPS, reciprocal, scale packed exps, DMA the
packed [1, 32*Q] vector out; the host scatters to the full [B, S-2]
grid (zeros where masked).
"""

import ml_dtypes
import numpy as np

import concourse.bass as bass
import concourse.mybir as mybir
import concourse.tile as tile
from concourse import bass_utils
from concourse.bass import ts

F32 = mybir.dt.float32
F32R = mybir.dt.float32r
BF16 = mybir.dt.bfloat16
AF = mybir.ActivationFunctionType

B, S, D, P = 256, 256, 1024, 512
SH = S - 2                  # head positions per batch
NCORES = 8
BC = B // NCORES            # 32 batches per core
KD = D // 128               # 8 k-tiles over D
KP = P // 128               # 4 k-tiles over P
EPS = 1e-7
NEG = -1e9

OPTS = {
    "s1_dtype": "bf16",   # stage-1 x and Wh: "f32r" | "bf16" | "fp8dr"
    "mm_dtype": "bf16",   # stages 2/3, scorer, bias: "f32r" | "bf16"
    "xr_bufs": 4,
    "c_bufs": 10,
    "ps_bufs": 8,
}

FP8 = mybir.dt.float8e4
_DT = {"f32r": F32R, "bf16": BF16, "f32": F32, "f16": mybir.dt.float16,
       "fp8dr": FP8}
_NPDT = {"f32r": np.float32, "bf16": ml_dtypes.bfloat16, "f32": np.float32,
         "f16": np.float16, "fp8dr": ml_dtypes.float8_e4m3}
WSCALE = 64.0  # fp8 weight pre-scale: U(-.05,.05) -> +-3.2, clear of subnormals


def _slot_plan(mask):
    """Per-rank slot sizes: each core's batches are sorted by unmasked
    count (descending) and slot k is sized to the max k-th-largest count
    over all cores (rounded up to 8).  Compile-time-uniform across cores,
    ~15% fewer rows than a single global max slot width."""
    n = np.asarray(mask)[:, :SH].sum(axis=1).reshape(NCORES, BC)
    srt = -np.sort(-n, axis=1)                  # [cores, BC] desc
    s = np.maximum(srt.max(axis=0), 1)
    s = ((s + 7) // 8 * 8).astype(int)
    assert s.max() <= 512
    return tuple(int(v) for v in s)


def _bounds(slots):
    b = [0]
    for s in slots:
        b.append(b[-1] + s)
    return b


def _segments(slots, nblk):
    """Per block: [(slot, lo, hi)] intersections of slot ranges with
    [512g, 512(g+1)), block-local coords."""
    bnd = _bounds(slots)
    segs = []
    for g in range(nblk):
        row = []
        for b in range(BC):
            lo = max(bnd[b], g * 512) - g * 512
            hi = min(bnd[b + 1], (g + 1) * 512) - g * 512
            if lo < hi:
                row.append((b, lo, hi))
        segs.append(row)
    return segs


# ---------------------------------------------------------------------------
# walrus in this container accepts at most ONE sync wait per instruction;
# split extra waits onto preceding NoOps on the same engine.
def _split_waits(nc, maxw=1):
    ctr = 0
    for f in nc.m.functions:
        for blk in f.blocks:
            insts = blk.instructions
            newlist = []
            changed = False
            for inst in insts:
                si = inst.sync_info
                if si is not None and len(si.on_wait) > maxw:
                    waits = list(si.on_wait)
                    keep = waits[len(waits) - maxw:]
                    extra = waits[: len(waits) - maxw]
                    for j in range(0, len(extra), maxw):
                        ctr += 1
                        newlist.append(
                            mybir.InstNoOp(
                                name=f"waitsplit-{ctr}",
                                engine=inst.engine,
                                ins=[],
                                outs=[],
                                sync_info=mybir.SyncInfo(
                                    on_wait=extra[j: j + maxw], on_update=[]
                                ),
                            )
                        )
                    inst.sync_info = mybir.SyncInfo(
                        on_wait=keep, on_update=list(si.on_update)
                    )
                    changed = True
                newlist.append(inst)
            if changed:
                insts[:] = newlist


# ---------------------------------------------------------------------------
def _build(opts=None, reps=1, slots=None):
    opts = dict(OPTS, **(opts or {}))
    nc = bass.Bass("TRN2", target_bir_lowering=False, debug=False)

    S1DT = _DT[opts["s1_dtype"]]
    MMDT = _DT[opts["mm_dtype"]]
    s1_fp8 = opts["s1_dtype"] == "fp8dr"
    slots = tuple(slots) if slots else (160,) * BC
    nblk = -(-sum(slots) // 512)
    rows = nblk * 512
    segs = _segments(slots, nblk)

    # One HWDGE queue (SP), explicitly ordered by need-time: consts,
    # xr[0], wp/wc stream, xr[1], w0/w1, then the in-loop xr stream.
    s1_dma = nc.sync
    mm_dma = nc.sync
    tail_dma = nc.sync

    xT_d = nc.dram_tensor("xT", [nblk, 128, KD * 512], S1DT,
                          kind="ExternalInput").ap()
    xp_d = nc.dram_tensor("xprep", [D, BC], MMDT, kind="ExternalInput").ap()
    xc_d = nc.dram_tensor("xchild", [D, BC], MMDT, kind="ExternalInput").ap()
    if s1_fp8:
        # Wh pre-packed for DoubleRow: [dk-pair, 128, jt*256 + o*128 + m]
        wh_d = nc.dram_tensor("wh", [KD // 2, 128, 2 * P], S1DT,
                              kind="ExternalInput").ap()
    else:
        wh_d = nc.dram_tensor("wh", [D, P], S1DT, kind="ExternalInput").ap()
    wp_d = nc.dram_tensor("wp", [D, P], MMDT, kind="ExternalInput").ap()
    wc_d = nc.dram_tensor("wc", [D, P], MMDT, kind="ExternalInput").ap()
    w0_d = nc.dram_tensor("w0", [P, P], MMDT, kind="ExternalInput").ap()
    w1_d = nc.dram_tensor("w1", [P, P], MMDT, kind="ExternalInput").ap()
    sc_d = nc.dram_tensor("scT", [128, KP], MMDT, kind="ExternalInput").ap()
    lm_d = nc.dram_tensor("lmask", [1, rows], F32, kind="ExternalInput").ap()
    out_d = nc.dram_tensor("out", [1, rows], F32, kind="ExternalOutput").ap()

    with tile.TileContext(nc) as tc:
        with (
            tc.tile_pool(name="consts", bufs=1) as consts,
            tc.tile_pool(name="ssb", bufs=3) as spool,
            tc.tile_pool(name="xr", bufs=opts["xr_bufs"]) as xpool,
            tc.tile_pool(name="acts", bufs=opts["c_bufs"]) as cpool,
            tc.tile_pool(name="ps", bufs=opts["ps_bufs"], space="PSUM") as pspool,
            tc.tile_pool(name="epi", bufs=1) as epi,
        ):
            # ---- constants -------------------------------------------------
            def load_packed(dram, k, n, dt, dma, tag):
                t = consts.tile([128, k * n], dt, tag=tag)
                dma.dma_start(
                    t[:].rearrange("p (k n) -> p k n", n=n),
                    dram.rearrange("(k p) n -> p k n", p=128),
                )
                return t

            wh_t = []
            if s1_fp8:
                for dkp in range(KD // 2):
                    wt = consts.tile([128, 2 * P], S1DT, tag=f"wh{dkp}",
                                     name=f"wh_t{dkp}")
                    mm_dma.dma_start(wt[:], wh_d[dkp])
                    wh_t.append(wt)
            else:
                for dk in range(KD):
                    wt = consts.tile([128, P], S1DT, tag=f"wh{dk}",
                                     name=f"wh_t{dk}")
                    mm_dma.dma_start(wt[:], wh_d[dk * 128: (dk + 1) * 128, :])
                    wh_t.append(wt)
            xp_r = load_packed(xp_d, KD, BC, MMDT, mm_dma, "xp")
            xc_r = load_packed(xc_d, KD, BC, MMDT, mm_dma, "xc")
            sc_r = consts.tile([128, KP], MMDT, tag="sc")
            mm_dma.dma_start(sc_r[:], sc_d[:])
            lm_sb = consts.tile([1, rows], F32, tag="lm")
            mm_dma.dma_start(lm_sb[:], lm_d[:])

            exps_all = epi.tile([1, rows], F32, tag="exps")
            outv = epi.tile([1, rows], F32, tag="outv")
            parts = epi.tile([1, 2 * BC], F32, tag="parts")
            sums = epi.tile([1, BC], F32, tag="sums")
            recips = epi.tile([1, BC], F32, tag="recips")
            s1scale = (1.0 / WSCALE) if s1_fp8 else 1.0
            bias_fm = []
            occ = {}

            def load_xr(g):
                xr = xpool.tile([128, KD * 512], S1DT, tag="xr")
                hw = KD * 512 // 2
                for h in range(2):
                    s1_dma.dma_start(
                        xr[:, h * hw: (h + 1) * hw],
                        xT_d[g, :, h * hw: (h + 1) * hw],
                    )
                return xr

            def s1_mms(g, xr):
                pss1s = []
                for jt in range(KP):
                    pss1 = pspool.tile([128, 512], F32, tag="ps",
                                       name=f"ps1_{g}_{jt}")
                    if s1_fp8:
                        for dkp in range(KD // 2):
                            nc.tensor.matmul(
                                pss1[:],
                                wh_t[dkp][:, jt * 256: (jt + 1) * 256]
                                .rearrange("p (o m) -> p o m", o=2),
                                xr[:, dkp * 1024: (dkp + 1) * 1024]
                                .rearrange("p (o n) -> p o n", o=2),
                                start=(dkp == 0),
                                stop=(dkp == KD // 2 - 1),
                                perf_mode=mybir.MatmulPerfMode.DoubleRow,
                            )
                    else:
                        for dk in range(KD):
                            nc.tensor.matmul(
                                pss1[:],
                                wh_t[dk][:, jt * 128: (jt + 1) * 128],
                                xr[:, dk * 512: (dk + 1) * 512],
                                start=(dk == 0),
                                stop=(dk == KD - 1),
                            )
                    pss1s.append(pss1)
                return pss1s

            def block_rest(g, pss1s):
                # per-batch bias added in-place on the (mostly idle) DVE
                # (gpsimd cannot access PSUM), then ONE full-tile tanh on
                # ACT: the ACT instruction has a ~293ns fixed cost, so
                # fewer, larger activations win.
                c_in = [None] * KP
                for jt in range(KP):
                    pss1 = pss1s[jt]
                    for (b, lo, hi) in segs[g]:
                        nc.vector.tensor_scalar_add(
                            pss1[:, lo:hi], pss1[:, lo:hi],
                            bias_fm[jt][:, b: b + 1],
                        )
                    ct = cpool.tile([128, 512], MMDT, tag="c1")
                    nc.scalar.activation(ct[:], pss1[:], AF.Tanh,
                                         scale=s1scale)
                    c_in[jt] = ct

                for stage, w_r in ((2, w0_r), (3, w1_r)):
                    c_out = [None] * KP
                    for qt in range(KP):
                        pss2 = pspool.tile([128, 512], F32, tag="ps",
                                           name=f"ps{stage}_{g}_{qt}")
                        for jk in range(KP):
                            nc.tensor.matmul(
                                pss2[:],
                                w_r[:, jk * P + qt * 128:
                                    jk * P + (qt + 1) * 128],
                                c_in[jk][:],
                                start=(jk == 0),
                                stop=(jk == KP - 1),
                            )
                        ct = cpool.tile([128, 512], MMDT, tag=f"c{stage}")
                        nc.scalar.activation(ct[:], pss2[:], AF.Tanh)
                        c_out[qt] = ct
                    c_in = c_out

                # scorer -> [1, 512] scores; pad log-mask added in-place on
                # DVE (PE is the bottleneck, DVE is nearly idle).
                pss = pspool.tile([1, 512], F32, tag="ps", name=f"pssc_{g}")
                for qk in range(KP):
                    nc.tensor.matmul(
                        pss[:],
                        sc_r[:, qk: qk + 1],
                        c_in[qk][:],
                        start=(qk == 0),
                        stop=(qk == KP - 1),
                    )
                nc.vector.tensor_add(
                    pss[0:1, :], pss[0:1, :],
                    lm_sb[0:1, g * 512: (g + 1) * 512],
                )
                nc.scalar.activation(
                    exps_all[0:1, g * 512: (g + 1) * 512],
                    pss[0:1, :],
                    AF.Exp,
                )
                # per-slot partial sums on DVE (pads contribute exp(-1e9)=0)
                for (b, lo, hi) in segs[g]:
                    k = occ.get(b, 0)
                    occ[b] = k + 1
                    assert k < 2, "slot spans >2 blocks"
                    nc.vector.tensor_reduce(
                        parts[0:1, k * BC + b: k * BC + b + 1],
                        exps_all[0:1, g * 512 + lo: g * 512 + hi],
                        mybir.AxisListType.X,
                        mybir.AluOpType.add,
                    )

            # ---- prologue, ordered by need-time ---------------------------
            # xr[0] + block-0 stage-1 MMs go FIRST (PE is FIFO: anything
            # emitted earlier that waits on a late DMA head-of-line blocks
            # the whole engine).  Then the bias-prep MMs (wp/wc stream),
            # then xr[1], then w0/w1.
            xr0 = load_xr(0)
            pre_pss1 = s1_mms(0, xr0)

            psbs = [pspool.tile([128, BC], F32, tag="ps", name=f"psb_{jt}")
                    for jt in range(KP)]
            for i, (xs, w_d) in enumerate(((xp_r, wp_d), (xc_r, wc_d))):
                for dk in range(KD):
                    wst = spool.tile([128, P], MMDT, tag="wst")
                    mm_dma.dma_start(wst[:], w_d[dk * 128: (dk + 1) * 128, :])
                    for jt in range(KP):
                        nc.tensor.matmul(
                            psbs[jt][:],
                            wst[:, jt * 128: (jt + 1) * 128],
                            xs[:, dk * BC: (dk + 1) * BC],
                            start=(i == 0 and dk == 0),
                            stop=(i == 1 and dk == KD - 1),
                        )
            for jt in range(KP):
                bf = consts.tile([128, BC], F32, tag=f"bias{jt}")
                if s1_fp8:
                    # bias is now added pre-scale: tanh(scale*(psum + b*WS))
                    nc.vector.tensor_scalar_mul(bf[:], psbs[jt][:], WSCALE)
                else:
                    nc.vector.tensor_copy(bf[:], psbs[jt][:])
                bias_fm.append(bf)

            xr1 = load_xr(1) if nblk > 1 else None
            w0_r = load_packed(w0_d, KP, P, MMDT, mm_dma, "w0")
            w1_r = load_packed(w1_d, KP, P, MMDT, mm_dma, "w1")

            # ---- main loop -------------------------------------------------
            for _rep in range(reps):
                occ.clear()
                nc.gpsimd.memset(parts[:], 0.0)
                for g in range(nblk):
                    if _rep == 0 and g == 0:
                        pss1s = pre_pss1
                    else:
                        if _rep == 0 and g == 1:
                            xr = xr1
                        else:
                            xr = load_xr(g)
                        pss1s = s1_mms(g, xr)
                    block_rest(g, pss1s)

                # ---- tail: combine partials, normalize, scale, DMA --------
                nc.vector.tensor_add(sums[:], parts[0:1, 0:BC],
                                     parts[0:1, BC: 2 * BC])
                nc.vector.tensor_scalar_add(sums[:], sums[:], EPS)
                nc.vector.reciprocal(recips[:], sums[:])
                # scalings on the idle Pool engine (all-SBUF operands) so
                # the DVE stays free for the next rep's bias adds
                for g in range(nblk):
                    for (b, lo, hi) in segs[g]:
                        nc.gpsimd.tensor_scalar_mul(
                            outv[0:1, g * 512 + lo: g * 512 + hi],
                            exps_all[0:1, g * 512 + lo: g * 512 + hi],
                            recips[0:1, b: b + 1],
                        )
                tail_dma.dma_start(out_d[:], outv[:])

    _split_waits(nc)
    return nc


# ---------------------------------------------------------------------------
class _Plan:
    def __init__(self, slots, idx_lists, orders):
        self.slots = slots
        self.bounds = _bounds(slots)
        self.nblk = -(-sum(slots) // 512)
        self.idx_lists = idx_lists  # [B] arrays of unmasked positions
        self.orders = orders        # [cores][BC] slot -> local batch index


def _host_prep(x, proj_head, proj_prep, proj_child, hidden_layers, scorer,
               mask, opts=None):
    opts = dict(OPTS, **(opts or {}))
    s1_np = _NPDT[opts["s1_dtype"]]
    mm_np = _NPDT[opts["mm_dtype"]]
    x = np.asarray(x, np.float32)
    mask = np.asarray(mask)
    idx_lists = [np.nonzero(mask[b, :SH])[0] for b in range(B)]
    nb = np.array([len(i) for i in idx_lists]).reshape(NCORES, BC)
    orders = [list(np.argsort(-nb[c], kind="stable")) for c in range(NCORES)]
    slots = _slot_plan(mask)
    bounds = _bounds(slots)
    nblk = -(-sum(slots) // 512)
    rows = nblk * 512

    s1_fp8 = opts["s1_dtype"] == "fp8dr"
    whf = np.asarray(proj_head, np.float32)
    if s1_fp8:
        wh = np.ascontiguousarray(
            np.clip(whf * WSCALE, -240, 240)
            .reshape(KD // 2, 2, 128, KP, 128).transpose(0, 2, 3, 1, 4)
            .reshape(KD // 2, 128, 2 * P).astype(s1_np)
        )
    else:
        wh = np.ascontiguousarray(whf.astype(s1_np))
    wp = np.ascontiguousarray(np.asarray(proj_prep, mm_np))
    wc = np.ascontiguousarray(np.asarray(proj_child, mm_np))
    hl = np.asarray(hidden_layers, np.float32)
    w0 = np.ascontiguousarray(hl[0].astype(mm_np))
    w1 = np.ascontiguousarray(hl[1].astype(mm_np))
    scT = np.ascontiguousarray(
        np.asarray(scorer, np.float32).reshape(KP, 128).T.astype(mm_np)
    )  # [128, 4]

    in_maps = []
    for c in range(NCORES):
        xb = x[c * BC: (c + 1) * BC]                        # [32, 256, 1024]
        packed = np.zeros((rows, D), np.float32)
        lm = np.full((1, rows), NEG, np.float32)
        for k in range(BC):                # slot k holds batch orders[c][k]
            b = orders[c][k]
            idx = idx_lists[c * BC + b]
            n = len(idx)
            assert n <= slots[k]
            packed[bounds[k]: bounds[k] + n] = xb[b, idx, :]
            lm[0, bounds[k]: bounds[k] + n] = 0.0
        if s1_fp8:
            xTc = np.ascontiguousarray(
                packed.reshape(nblk, 512, KD // 2, 2, 128)
                .transpose(0, 4, 2, 3, 1).astype(s1_np)
            ).reshape(nblk, 128, KD * 512)
        else:
            xTc = np.ascontiguousarray(
                packed.reshape(nblk, 512, KD, 128).transpose(0, 3, 2, 1)
                .astype(s1_np)
            ).reshape(nblk, 128, KD * 512)
        # prep/child encodings in SLOT order so bias_fm columns line up
        xpc = np.ascontiguousarray(
            xb[orders[c], S - 2, :].T.astype(mm_np))             # [1024, 32]
        xcc = np.ascontiguousarray(
            xb[orders[c], S - 1, :].T.astype(mm_np))             # [1024, 32]
        in_maps.append(
            {
                "xT": xTc, "xprep": xpc, "xchild": xcc,
                "wh": wh, "wp": wp, "wc": wc, "w0": w0, "w1": w1,
                "scT": scT, "lmask": lm,
            }
        )
    return in_maps, _Plan(slots, idx_lists, orders)


_NC_CACHE = {}


def _get_nc(slots, opts=None, reps=1):
    key = (tuple(slots), reps, tuple(sorted((opts or {}).items())))
    if key not in _NC_CACHE:
        _NC_CACHE[key] = _build(opts=opts, reps=reps, slots=slots)
    return _NC_CACHE[key]


def kernel(x, proj_head, proj_prep, proj_child, hidden_layers, scorer, mask,
           opts=None):
    in_maps, plan = _host_prep(
        x, proj_head, proj_prep, proj_child, hidden_layers, scorer, mask,
        opts=opts,
    )
    nc = _get_nc(plan.slots, opts=opts)
    res = bass_utils.run_bass_kernel_spmd(
        nc, in_maps, core_ids=list(range(NCORES))
    )
    out = np.zeros((B, SH), np.float32)
    for c in range(NCORES):
        vals = res.results[c]["out"][0]         # [1, rows] packed
        for k in range(BC):
            b = plan.orders[c][k]
            idx = plan.idx_lists[c * BC + b]
            out[c * BC + b, idx] = vals[plan.bounds[k]:
                                        plan.bounds[k] + len(idx)]
    return out


if __name__ == "__main__":
    rng = np.random.default_rng(0)
    x = rng.standard_normal((B, S, D)).astype(np.float32)
    u = lambda shp: rng.uniform(-0.05, 0.05, shp).astype(np.float32)
    inputs = dict(
        x=x, proj_head=u((D, P)), proj_prep=u((D, P)), proj_child=u((D, P)),
        hidden_layers=u((2, P, P)), scorer=u((P,)),
        mask=rng.integers(0, 2, (B, S)).astype(bool),
    )
    out = kernel(**inputs)
    print("kernel out", out.shape, out.dtype, out[:2, :4])


# revision 58
# speedup vs baseline: 1.3981x; 1.2214x over previous
"""Trainium2 Bass kernel for nn_AttachmentPredictor.

Pipeline (per core, data-parallel over batch; 32 batches/core).

Sparsity: the reference zeroes every output where mask=0, so only
unmasked head positions (~50%) need scores at all.  The host sorts each
core's batches by unmasked count (descending) and packs batch rank k
into a slot sized to the max k-th-largest count over all cores (rounded
to 8) — compile-time-uniform across cores (SPMD) yet ~18% tighter than
one global slot width (typ. ~4224 rows/core = 9 blocks vs 8192 dense).
Pad columns carry -1e9 in a log-mask so exp() kills them.  The NEFF is
(re)compiled per slot plan and cached; any mask distribution works.

Per 512-row block:
  stage1: head projection, feature-major psum[jt] += Wh[dk,jt] @ xT[dk,:]
  tanh(Y1 + bias) -> c1, with per-(batch-slot x block) activation
  segments supplying the per-batch prep+child bias (all 32-aligned)
  stage2/3: hidden layers, feature-major, tanh -> c2, c3
  scorer: [1,512] psum rows of scores via M=1 matmuls
  epilogue: scores + logmask, exp per slot segment with accumulated
  per-slot partial sums.
Tail: combine partials, +EPS, reciprocal, scale packed exps, DMA the
packed [1, 32*Q] vector out; the host scatters to the full [B, S-2]
grid (zeros where masked).
"""

import ml_dtypes
import numpy as np

import concourse.bass as bass
import concourse.mybir as mybir
import concourse.tile as tile
from concourse import bass_utils
from concourse.bass import ts

F32 = mybir.dt.float32
F32R = mybir.dt.float32r
BF16 = mybir.dt.bfloat16
AF = mybir.ActivationFunctionType

B, S, D, P = 256, 256, 1024, 512
SH = S - 2                  # head positions per batch
NCORES = 8
BC = B // NCORES            # 32 batches per core
KD = D // 128               # 8 k-tiles over D
KP = P // 128               # 4 k-tiles over P
EPS = 1e-7
NEG = -1e9

OPTS = {
    "s1_dtype": "bf16",   # stage-1 x and Wh: "f32r" | "bf16" | "fp8dr"
    "mm_dtype": "bf16",   # stages 2/3, scorer, bias: "f32r" | "bf16"
    "xr_bufs": 4,
    "c_bufs": 10,
    "ps_bufs": 8,
}

FP8 = mybir.dt.float8e4
_DT = {"f32r": F32R, "bf16": BF16, "f32": F32, "f16": mybir.dt.float16,
       "fp8dr": FP8}
_NPDT = {"f32r": np.float32, "bf16": ml_dtypes.bfloat16, "f32": np.float32,
         "f16": np.float16, "fp8dr": ml_dtypes.float8_e4m3}
WSCALE = 64.0  # fp8 weight pre-scale: U(-.05,.05) -> +-3.2, clear of subnormals


def _slot_plan(mask):
    """Per-rank slot sizes: each core's batches are sorted by unmasked
    count (descending) and slot k is sized to the max k-th-largest count
    over all cores (rounded up to 8).  Compile-time-uniform across cores,
    ~15% fewer rows than a single global max slot width."""
    n = np.asarray(mask)[:, :SH].sum(axis=1).reshape(NCORES, BC)
    srt = -np.sort(-n, axis=1)                  # [cores, BC] desc
    s = np.maximum(srt.max(axis=0), 1)
    s = ((s + 7) // 8 * 8).astype(int)
    assert s.max() <= 512
    return tuple(int(v) for v in s)


def _bounds(slots):
    b = [0]
    for s in slots:
        b.append(b[-1] + s)
    return b


def _blocks(slots):
    """Variable-width blocks: full 512-col blocks plus one narrow tail
    block (rounded to 8) — tail matmuls cost ~N cycles, so a mostly-pad
    512-wide tail would waste ~3/4 of a block of PE time."""
    tot = sum(slots)
    widths = [512] * (tot // 512)
    rem = tot - 512 * len(widths)
    if rem:
        widths.append(-(-rem // 8) * 8)
    starts = [0]
    for w in widths:
        starts.append(starts[-1] + w)
    return widths, starts[:-1]


def _segments(slots, widths, starts):
    """Per block: [(slot, lo, hi)] intersections of slot ranges with
    [starts[g], starts[g]+widths[g]), block-local coords."""
    bnd = _bounds(slots)
    segs = []
    for g, (w, s0) in enumerate(zip(widths, starts)):
        row = []
        for b in range(BC):
            lo = max(bnd[b], s0) - s0
            hi = min(bnd[b + 1], s0 + w) - s0
            if lo < hi:
                row.append((b, lo, hi))
        segs.append(row)
    return segs


# ---------------------------------------------------------------------------
# walrus in this container accepts at most ONE sync wait per instruction;
# split extra waits onto preceding NoOps on the same engine.
def _split_waits(nc, maxw=1):
    ctr = 0
    for f in nc.m.functions:
        for blk in f.blocks:
            insts = blk.instructions
            newlist = []
            changed = False
            for inst in insts:
                si = inst.sync_info
                if si is not None and len(si.on_wait) > maxw:
                    waits = list(si.on_wait)
                    keep = waits[len(waits) - maxw:]
                    extra = waits[: len(waits) - maxw]
                    for j in range(0, len(extra), maxw):
                        ctr += 1
                        newlist.append(
                            mybir.InstNoOp(
                                name=f"waitsplit-{ctr}",
                                engine=inst.engine,
                                ins=[],
                                outs=[],
                                sync_info=mybir.SyncInfo(
                                    on_wait=extra[j: j + maxw], on_update=[]
                                ),
                            )
                        )
                    inst.sync_info = mybir.SyncInfo(
                        on_wait=keep, on_update=list(si.on_update)
                    )
                    changed = True
                newlist.append(inst)
            if changed:
                insts[:] = newlist


# ---------------------------------------------------------------------------
def _build(opts=None, reps=1, slots=None):
    opts = dict(OPTS, **(opts or {}))
    nc = bass.Bass("TRN2", target_bir_lowering=False, debug=False)

    S1DT = _DT[opts["s1_dtype"]]
    MMDT = _DT[opts["mm_dtype"]]
    s1_fp8 = opts["s1_dtype"] == "fp8dr"
    slots = tuple(slots) if slots else (160,) * BC
    widths, starts = _blocks(slots)
    nblk = len(widths)
    rows = starts[-1] + widths[-1]
    segs = _segments(slots, widths, starts)

    # One HWDGE queue (SP), explicitly ordered by need-time: consts,
    # xr[0], wp/wc stream, xr[1], w0/w1, then the in-loop xr stream.
    s1_dma = nc.sync
    mm_dma = nc.sync
    tail_dma = nc.sync

    xT_d = nc.dram_tensor("xT", [128, KD * rows], S1DT,
                          kind="ExternalInput").ap()
    xp_d = nc.dram_tensor("xprep", [D, BC], MMDT, kind="ExternalInput").ap()
    xc_d = nc.dram_tensor("xchild", [D, BC], MMDT, kind="ExternalInput").ap()
    if s1_fp8:
        # Wh pre-packed for DoubleRow: [dk-pair, 128, jt*256 + o*128 + m]
        wh_d = nc.dram_tensor("wh", [KD // 2, 128, 2 * P], S1DT,
                              kind="ExternalInput").ap()
    else:
        wh_d = nc.dram_tensor("wh", [D, P], S1DT, kind="ExternalInput").ap()
    wp_d = nc.dram_tensor("wp", [D, P], MMDT, kind="ExternalInput").ap()
    wc_d = nc.dram_tensor("wc", [D, P], MMDT, kind="ExternalInput").ap()
    w0_d = nc.dram_tensor("w0", [P, P], MMDT, kind="ExternalInput").ap()
    w1_d = nc.dram_tensor("w1", [P, P], MMDT, kind="ExternalInput").ap()
    sc_d = nc.dram_tensor("scT", [128, KP], MMDT, kind="ExternalInput").ap()
    lm_d = nc.dram_tensor("lmask", [1, rows], F32, kind="ExternalInput").ap()
    out_d = nc.dram_tensor("out", [1, rows], F32, kind="ExternalOutput").ap()

    with tile.TileContext(nc) as tc:
        with (
            tc.tile_pool(name="consts", bufs=1) as consts,
            tc.tile_pool(name="ssb", bufs=3) as spool,
            tc.tile_pool(name="xr", bufs=opts["xr_bufs"]) as xpool,
            tc.tile_pool(name="acts", bufs=opts["c_bufs"]) as cpool,
            tc.tile_pool(name="ps", bufs=opts["ps_bufs"], space="PSUM") as pspool,
            tc.tile_pool(name="epi", bufs=1) as epi,
        ):
            # ---- constants -------------------------------------------------
            def load_packed(dram, k, n, dt, dma, tag):
                t = consts.tile([128, k * n], dt, tag=tag)
                dma.dma_start(
                    t[:].rearrange("p (k n) -> p k n", n=n),
                    dram.rearrange("(k p) n -> p k n", p=128),
                )
                return t

            wh_t = []
            if s1_fp8:
                for dkp in range(KD // 2):
                    wt = consts.tile([128, 2 * P], S1DT, tag=f"wh{dkp}",
                                     name=f"wh_t{dkp}")
                    mm_dma.dma_start(wt[:], wh_d[dkp])
                    wh_t.append(wt)
            else:
                for dk in range(KD):
                    wt = consts.tile([128, P], S1DT, tag=f"wh{dk}",
                                     name=f"wh_t{dk}")
                    mm_dma.dma_start(wt[:], wh_d[dk * 128: (dk + 1) * 128, :])
                    wh_t.append(wt)
            xp_r = load_packed(xp_d, KD, BC, MMDT, mm_dma, "xp")
            xc_r = load_packed(xc_d, KD, BC, MMDT, mm_dma, "xc")
            sc_r = consts.tile([128, KP], MMDT, tag="sc")
            mm_dma.dma_start(sc_r[:], sc_d[:])
            lm_sb = consts.tile([1, rows], F32, tag="lm")
            mm_dma.dma_start(lm_sb[:], lm_d[:])

            exps_all = epi.tile([1, rows], F32, tag="exps")
            outv = epi.tile([1, rows], F32, tag="outv")
            parts = epi.tile([1, 2 * BC], F32, tag="parts")
            sums = epi.tile([1, BC], F32, tag="sums")
            recips = epi.tile([1, BC], F32, tag="recips")
            s1scale = (1.0 / WSCALE) if s1_fp8 else 1.0
            bias_fm = []
            occ = {}

            def load_xr(g):
                w = widths[g]
                base = KD * starts[g]
                xr = xpool.tile([128, KD * w], S1DT, tag="xr")
                hw = KD * w // 2
                for h in range(2):
                    s1_dma.dma_start(
                        xr[:, h * hw: (h + 1) * hw],
                        xT_d[:, base + h * hw: base + (h + 1) * hw],
                    )
                return xr

            def s1_mms(g, xr):
                w = widths[g]
                pss1s = []
                for jt in range(KP):
                    pss1 = pspool.tile([128, w], F32, tag="ps",
                                       name=f"ps1_{g}_{jt}")
                    if s1_fp8:
                        for dkp in range(KD // 2):
                            nc.tensor.matmul(
                                pss1[:],
                                wh_t[dkp][:, jt * 256: (jt + 1) * 256]
                                .rearrange("p (o m) -> p o m", o=2),
                                xr[:, dkp * 2 * w: (dkp + 1) * 2 * w]
                                .rearrange("p (o n) -> p o n", o=2),
                                start=(dkp == 0),
                                stop=(dkp == KD // 2 - 1),
                                perf_mode=mybir.MatmulPerfMode.DoubleRow,
                            )
                    else:
                        for dk in range(KD):
                            nc.tensor.matmul(
                                pss1[:],
                                wh_t[dk][:, jt * 128: (jt + 1) * 128],
                                xr[:, dk * w: (dk + 1) * w],
                                start=(dk == 0),
                                stop=(dk == KD - 1),
                            )
                    pss1s.append(pss1)
                return pss1s

            def s1_finish(g, pss1s):
                # per-batch bias added in-place on the (mostly idle) DVE
                # (gpsimd cannot access PSUM), then ONE full-tile tanh on
                # ACT: the ACT instruction has a ~293ns fixed cost, so
                # fewer, larger activations win.
                w = widths[g]
                c1 = [None] * KP
                for jt in range(KP):
                    pss1 = pss1s[jt]
                    for (b, lo, hi) in segs[g]:
                        nc.vector.tensor_scalar_add(
                            pss1[:, lo:hi], pss1[:, lo:hi],
                            bias_fm[jt][:, b: b + 1],
                        )
                    ct = cpool.tile([128, w], MMDT, tag="c1")
                    nc.scalar.activation(ct[:], pss1[:], AF.Tanh,
                                         scale=s1scale)
                    c1[jt] = ct
                return c1

            def block_rest(g, c_in):
                w = widths[g]
                s0 = starts[g]
                for stage, w_r in ((2, w0_r), (3, w1_r)):
                    c_out = [None] * KP
                    for qt in range(KP):
                        pss2 = pspool.tile([128, w], F32, tag="ps",
                                           name=f"ps{stage}_{g}_{qt}")
                        for jk in range(KP):
                            nc.tensor.matmul(
                                pss2[:],
                                w_r[:, jk * P + qt * 128:
                                    jk * P + (qt + 1) * 128],
                                c_in[jk][:],
                                start=(jk == 0),
                                stop=(jk == KP - 1),
                            )
                        ct = cpool.tile([128, w], MMDT, tag=f"c{stage}")
                        nc.scalar.activation(ct[:], pss2[:], AF.Tanh)
                        c_out[qt] = ct
                    c_in = c_out

                # scorer -> [1, w] scores; pad log-mask added in-place on
                # DVE (PE is the bottleneck, DVE is nearly idle).
                pss = pspool.tile([1, w], F32, tag="ps", name=f"pssc_{g}")
                for qk in range(KP):
                    nc.tensor.matmul(
                        pss[:],
                        sc_r[:, qk: qk + 1],
                        c_in[qk][:],
                        start=(qk == 0),
                        stop=(qk == KP - 1),
                    )
                nc.vector.tensor_add(
                    pss[0:1, :], pss[0:1, :],
                    lm_sb[0:1, s0: s0 + w],
                )
                nc.scalar.activation(
                    exps_all[0:1, s0: s0 + w],
                    pss[0:1, :],
                    AF.Exp,
                )
                # per-slot partial sums on DVE (pads contribute exp(-1e9)=0)
                for (b, lo, hi) in segs[g]:
                    k = occ.get(b, 0)
                    occ[b] = k + 1
                    assert k < 2, "slot spans >2 blocks"
                    nc.vector.tensor_reduce(
                        parts[0:1, k * BC + b: k * BC + b + 1],
                        exps_all[0:1, s0 + lo: s0 + hi],
                        mybir.AxisListType.X,
                        mybir.AluOpType.add,
                    )

            # ---- prologue, ordered by need-time ---------------------------
            # xr[0] + block-0 stage-1 MMs go FIRST (PE is FIFO: anything
            # emitted earlier that waits on a late DMA head-of-line blocks
            # the whole engine).  Then the bias-prep MMs (wp/wc stream),
            # then xr[1], then w0/w1.
            xr0 = load_xr(0)
            pre_pss1 = s1_mms(0, xr0)

            psbs = [pspool.tile([128, BC], F32, tag="ps", name=f"psb_{jt}")
                    for jt in range(KP)]
            for i, (xs, w_d) in enumerate(((xp_r, wp_d), (xc_r, wc_d))):
                for dk in range(KD):
                    wst = spool.tile([128, P], MMDT, tag="wst")
                    mm_dma.dma_start(wst[:], w_d[dk * 128: (dk + 1) * 128, :])
                    for jt in range(KP):
                        nc.tensor.matmul(
                            psbs[jt][:],
                            wst[:, jt * 128: (jt + 1) * 128],
                            xs[:, dk * BC: (dk + 1) * BC],
                            start=(i == 0 and dk == 0),
                            stop=(i == 1 and dk == KD - 1),
                        )
            for jt in range(KP):
                bf = consts.tile([128, BC], F32, tag=f"bias{jt}")
                if s1_fp8:
                    # bias is now added pre-scale: tanh(scale*(psum + b*WS))
                    nc.vector.tensor_scalar_mul(bf[:], psbs[jt][:], WSCALE)
                else:
                    nc.vector.tensor_copy(bf[:], psbs[jt][:])
                bias_fm.append(bf)

            xr1 = load_xr(1) if nblk > 1 else None
            w0_r = load_packed(w0_d, KP, P, MMDT, mm_dma, "w0")
            w1_r = load_packed(w1_d, KP, P, MMDT, mm_dma, "w1")

            # ---- main loop, software-pipelined ------------------------------
            # Block g+1's stage-1 MMs + bias + tanh are emitted before block
            # g's stage-2/3 so the PE never idles on the c1 tanh latency.
            # PSUM stays within 8 banks: s1(g+1) holds 4 while s2(g) uses
            # the 4 freed by c1(g)'s tanh.
            for _rep in range(reps):
                occ.clear()
                nc.gpsimd.memset(parts[:], 0.0)
                c1_cur = None
                for g in range(nblk):
                    if c1_cur is None:
                        pss1s = pre_pss1 if (_rep == 0 and g == 0) \
                            else s1_mms(g, load_xr(g))
                        c1_cur = s1_finish(g, pss1s)
                    if g + 1 < nblk:
                        xr = xr1 if (_rep == 0 and g + 1 == 1) \
                            else load_xr(g + 1)
                        c1_next = s1_finish(g + 1, s1_mms(g + 1, xr))
                    else:
                        c1_next = None
                    block_rest(g, c1_cur)
                    c1_cur = c1_next

                # ---- tail: combine partials, normalize, scale, DMA --------
                nc.vector.tensor_add(sums[:], parts[0:1, 0:BC],
                                     parts[0:1, BC: 2 * BC])
                nc.vector.tensor_scalar_add(sums[:], sums[:], EPS)
                nc.vector.reciprocal(recips[:], sums[:])
                # scalings on the idle Pool engine (all-SBUF operands) so
                # the DVE stays free for the next rep's bias adds
                for g in range(nblk):
                    for (b, lo, hi) in segs[g]:
                        nc.gpsimd.tensor_scalar_mul(
                            outv[0:1, starts[g] + lo: starts[g] + hi],
                            exps_all[0:1, starts[g] + lo: starts[g] + hi],
                            recips[0:1, b: b + 1],
                        )
                tail_dma.dma_start(out_d[:], outv[:])

    _split_waits(nc)
    return nc


# ---------------------------------------------------------------------------
class _Plan:
    def __init__(self, slots, idx_lists, orders):
        self.slots = slots
        self.bounds = _bounds(slots)
        self.nblk = -(-sum(slots) // 512)
        self.idx_lists = idx_lists  # [B] arrays of unmasked positions
        self.orders = orders        # [cores][BC] slot -> local batch index


def _host_prep(x, proj_head, proj_prep, proj_child, hidden_layers, scorer,
               mask, opts=None):
    opts = dict(OPTS, **(opts or {}))
    s1_np = _NPDT[opts["s1_dtype"]]
    mm_np = _NPDT[opts["mm_dtype"]]
    x = np.asarray(x, np.float32)
    mask = np.asarray(mask)
    idx_lists = [np.nonzero(mask[b, :SH])[0] for b in range(B)]
    nb = np.array([len(i) for i in idx_lists]).reshape(NCORES, BC)
    orders = [list(np.argsort(-nb[c], kind="stable")) for c in range(NCORES)]
    slots = _slot_plan(mask)
    bounds = _bounds(slots)
    widths, starts = _blocks(slots)
    rows = starts[-1] + widths[-1]

    s1_fp8 = opts["s1_dtype"] == "fp8dr"
    whf = np.asarray(proj_head, np.float32)
    if s1_fp8:
        wh = np.ascontiguousarray(
            np.clip(whf * WSCALE, -240, 240)
            .reshape(KD // 2, 2, 128, KP, 128).transpose(0, 2, 3, 1, 4)
            .reshape(KD // 2, 128, 2 * P).astype(s1_np)
        )
    else:
        wh = np.ascontiguousarray(whf.astype(s1_np))
    wp = np.ascontiguousarray(np.asarray(proj_prep, mm_np))
    wc = np.ascontiguousarray(np.asarray(proj_child, mm_np))
    hl = np.asarray(hidden_layers, np.float32)
    w0 = np.ascontiguousarray(hl[0].astype(mm_np))
    w1 = np.ascontiguousarray(hl[1].astype(mm_np))
    scT = np.ascontiguousarray(
        np.asarray(scorer, np.float32).reshape(KP, 128).T.astype(mm_np)
    )  # [128, 4]

    in_maps = []
    for c in range(NCORES):
        xb = x[c * BC: (c + 1) * BC]                        # [32, 256, 1024]
        packed = np.zeros((rows, D), np.float32)
        lm = np.full((1, rows), NEG, np.float32)
        for k in range(BC):                # slot k holds batch orders[c][k]
            b = orders[c][k]
            idx = idx_lists[c * BC + b]
            n = len(idx)
            assert n <= slots[k]
            packed[bounds[k]: bounds[k] + n] = xb[b, idx, :]
            lm[0, bounds[k]: bounds[k] + n] = 0.0
        chunks = []
        for w, s0 in zip(widths, starts):
            blk = packed[s0: s0 + w]                        # [w, D]
            if s1_fp8:
                chunks.append(
                    blk.reshape(w, KD // 2, 2, 128)
                    .transpose(3, 1, 2, 0).reshape(128, KD * w)
                )
            else:
                chunks.append(
                    blk.reshape(w, KD, 128).transpose(2, 1, 0)
                    .reshape(128, KD * w)
                )
        xTc = np.ascontiguousarray(
            np.concatenate(chunks, axis=1).astype(s1_np))   # [128, KD*rows]
        # prep/child encodings in SLOT order so bias_fm columns line up
        xpc = np.ascontiguousarray(
            xb[orders[c], S - 2, :].T.astype(mm_np))             # [1024, 32]
        xcc = np.ascontiguousarray(
            xb[orders[c], S - 1, :].T.astype(mm_np))             # [1024, 32]
        in_maps.append(
            {
                "xT": xTc, "xprep": xpc, "xchild": xcc,
                "wh": wh, "wp": wp, "wc": wc, "w0": w0, "w1": w1,
                "scT": scT, "lmask": lm,
            }
        )
    return in_maps, _Plan(slots, idx_lists, orders)


_NC_CACHE = {}


def _get_nc(slots, opts=None, reps=1):
    key = (tuple(slots), reps, tuple(sorted((opts or {}).items())))
    if key not in _NC_CACHE:
        _NC_CACHE[key] = _build(opts=opts, reps=reps, slots=slots)
    return _NC_CACHE[key]


def kernel(x, proj_head, proj_prep, proj_child, hidden_layers, scorer, mask,
           opts=None):
    in_maps, plan = _host_prep(
        x, proj_head, proj_prep, proj_child, hidden_layers, scorer, mask,
        opts=opts,
    )
    nc = _get_nc(plan.slots, opts=opts)
    res = bass_utils.run_bass_kernel_spmd(
        nc, in_maps, core_ids=list(range(NCORES))
    )
    out = np.zeros((B, SH), np.float32)
    for c in range(NCORES):
        vals = res.results[c]["out"][0]         # [1, rows] packed
        for k in range(BC):
            b = plan.orders[c][k]
            idx = plan.idx_lists[c * BC + b]
            out[c * BC + b, idx] = vals[plan.bounds[k]:
                                        plan.bounds[k] + len(idx)]
    return out


if __name__ == "__main__":
    rng = np.random.default_rng(0)
    x = rng.standard_normal((B, S, D)).astype(np.float32)
    u = lambda shp: rng.uniform(-0.05, 0.05, shp).astype(np.float32)
    inputs = dict(
        x=x, proj_head=u((D, P)), proj_prep=u((D, P)), proj_child=u((D, P)),
        hidden_layers=u((2, P, P)), scorer=u((P,)),
        mask=rng.integers(0, 2, (B, S)).astype(bool),
    )
    out = kernel(**inputs)
    print("kernel out", out.shape, out.dtype, out[:2, :4])
